# revision 1
# baseline (speedup 1.0000x reference)
"""Performer (FAVOR+) multi-head fast-attention TRN2 kernel — self-contained.

Problem: B=4, N=4096, D=1024, H=16, M=256, DH=64.
Sharding: 2 heads per core (head-parallel attention) on 8 NeuronCores;
on-device AllToAll re-shards to sequence-parallel for the output Linear
(row-parallel, no partial sums); host stitches the 8 n-shards.

All Performer stabilizers that cancel in the num/den ratio are dropped
on device; the k-side row max and ||k||^2 factors are folded into v, so
the result matches the reference exactly up to float rounding.
"""
import contextlib
import sys

sys.path.insert(0, "/opt/trn_rl_repo")

import numpy as np

import concourse.bacc as bacc
import concourse.mybir as mybir
from concourse.tile import TileContext
from concourse.bass_utils import run_bass_kernel_spmd

F32 = mybir.dt.float32
F32R = mybir.dt.float32r
AF = mybir.ActivationFunctionType
ALU = mybir.AluOpType

NCORES = 8
B, N, D = 4, 4096, 1024
H, M, DH = 16, 256, 64
T = N // 128
J = N // 512
NS = N // NCORES
DS = float(DH) ** -0.25

_CACHE = {}


def _build():
    nc = bacc.Bacc(num_devices=NCORES)
    groups = [list(range(NCORES))]

    qT = nc.declare_dram_parameter("qT", [B, 2, DH, N], F32, isOutput=False)
    kT = nc.declare_dram_parameter("kT", [B, 2, DH, N], F32, isOutput=False)
    kn = nc.declare_dram_parameter("kn", [B, 128, T, 128], F32, isOutput=False)
    vn = nc.declare_dram_parameter("vn", [B, 128, T, 128], F32, isOutput=False)
    projT2 = nc.declare_dram_parameter("projT2", [128, M], F32, isOutput=False)
    WT = nc.declare_dram_parameter("WT", [D, D], F32, isOutput=False)
    ident = nc.declare_dram_parameter("ident", [128, 128], F32, isOutput=False)
    out_ext = nc.declare_dram_parameter("out", [B, NS, D], F32, isOutput=True)

    h_in = nc.dram_tensor("h_in", [B, NCORES, 130, NS], F32)
    h_out = nc.dram_tensor("h_out", [B, NCORES, 130, NS], F32)
    dinv_scr = nc.dram_tensor("dinv_scr", [B, 2 * NCORES * NS], F32)
    den_scr = nc.dram_tensor("den_scr", [B, 2 * NCORES * NS], F32)

    with TileContext(nc) as tc:
        with contextlib.ExitStack() as stk:
            const_p = stk.enter_context(tc.tile_pool(name="const", bufs=1))
            qkT_p = stk.enter_context(tc.tile_pool(name="qkT", bufs=2))
            knv_p = stk.enter_context(tc.tile_pool(name="knv", bufs=1))
            ek_p = stk.enter_context(tc.tile_pool(name="ek", bufs=1))
            small_p = stk.enter_context(tc.tile_pool(name="small", bufs=3))
            vaug_p = stk.enter_context(tc.tile_pool(name="vaug", bufs=1))
            qpt_p = stk.enter_context(tc.tile_pool(name="qpt", bufs=3))
            stag_p = stk.enter_context(tc.tile_pool(name="stag", bufs=3))
            lin_p = stk.enter_context(tc.tile_pool(name="lin", bufs=1))
            outc_p = stk.enter_context(tc.tile_pool(name="outc", bufs=3))
            ps_k = stk.enter_context(tc.tile_pool(name="psk", bufs=1, space="PSUM"))
            ps_q = stk.enter_context(tc.tile_pool(name="psq", bufs=1, space="PSUM"))
            ps_ctx = stk.enter_context(tc.tile_pool(name="psctx", bufs=1, space="PSUM"))
            ps_o = stk.enter_context(tc.tile_pool(name="pso", bufs=2, space="PSUM"))
            ps_lin = stk.enter_context(tc.tile_pool(name="pslin", bufs=1, space="PSUM"))

            projT2_sb = const_p.tile([128, M], F32R, tag="projT2")
            nc.sync.dma_start(out=projT2_sb[:], in_=projT2[:].bitcast(F32R))
            ident_sb = const_p.tile([128, 128], F32, tag="ident")
            nc.sync.dma_start(out=ident_sb[:], in_=ident[:])
            WT_sb = const_p.tile([128, NCORES, D], F32R, tag="WT")
            nc.sync.dma_start(out=WT_sb[:],
                              in_=WT[:].rearrange("(cc p) o -> p cc o", p=128).bitcast(F32R))

            for b in range(B):
                kn_sb = knv_p.tile([128, T, 128], F32, tag="kn")
                nc.sync.dma_start(out=kn_sb[:], in_=kn[b])
                v_sb = knv_p.tile([128, T, 128], F32, tag="v")
                nc.sync.dma_start(out=v_sb[:], in_=vn[b])

                kflat = kn_sb[:].rearrange("p t d -> p (t d)")
                nc.gpsimd.tensor_tensor(out=kflat, in0=kflat, in1=kflat,
                                        op=ALU.mult)
                dn_raw = small_p.tile([128, T, 2], F32, tag="dn")
                nc.vector.tensor_reduce(
                    out=dn_raw[:],
                    in_=kn_sb[:].rearrange("p t (h d) -> p t h d", h=2),
                    axis=mybir.AxisListType.X, op=ALU.add)

                for h in range(2):
                    qkT_sb = qkT_p.tile([128, N], F32R, tag="qkT")
                    nc.sync.dma_start(out=qkT_sb[0:DH, :], in_=kT[b, h].bitcast(F32R))
                    nc.sync.dma_start(out=qkT_sb[DH:128, :], in_=qT[b, h].bitcast(F32R))

                    ek_sb = ek_p.tile([128, T, M], F32R, tag="ek")
                    me = small_p.tile([128, T], F32, tag="me")
                    for tb in range(T // 4):
                        pk4 = ps_k.tile([128, 4, M], F32, tag="pk")
                        for qq in range(4):
                            t = 4 * tb + qq
                            nc.tensor.matmul(
                                pk4[:, qq, :], qkT_sb[0:DH, 128 * t:128 * (t + 1)],
                                projT2_sb[0:DH, :],
                                start=True, stop=True, skip_group_check=True)
                        nc.scalar.activation(ek_sb[:, 4 * tb:4 * (tb + 1), :], pk4[:],
                                             AF.Exp, scale=DS)
                        nc.vector.tensor_reduce(
                            out=me[:, 4 * tb:4 * (tb + 1)],
                            in_=ek_sb[:, 4 * tb:4 * (tb + 1), :],
                            axis=mybir.AxisListType.X, op=ALU.max)
                    eg = small_p.tile([128, T], F32, tag="eg")
                    nc.scalar.activation(eg[:], dn_raw[:, :, h], AF.Exp,
                                         scale=-0.5 * DS * DS)
                    rme = small_p.tile([128, T], F32, tag="rme")
                    nc.vector.reciprocal(rme[:], me[:])
                    g = small_p.tile([128, T], F32, tag="g")
                    nc.vector.tensor_tensor(out=g[:], in0=eg[:], in1=rme[:],
                                            op=ALU.mult)

                    vaug = vaug_p.tile([128, T, 65], F32R, tag="vaug")
                    nc.gpsimd.tensor_tensor(
                        out=vaug[:, :, 0:DH], in0=v_sb[:, :, DH * h:DH * (h + 1)],
                        in1=g[:].rearrange("p (t one) -> p t one", one=1)
                             .broadcast_to([128, T, DH]),
                        op=ALU.mult)
                    nc.gpsimd.tensor_copy(vaug[:, :, DH], g[:])

                    pctx = ps_ctx.tile([65, M], F32, tag="pctx")
                    for t in range(T):
                        nc.tensor.matmul(
                            pctx[:], vaug[:, t, :],
                            ek_sb[:, t, :],
                            start=(t == 0), stop=(t == T - 1), skip_group_check=True)
                    ctxs = small_p.tile([65, M], F32, tag="ctxs")
                    nc.vector.tensor_copy(ctxs[:], pctx[:])

                    ctxT = small_p.tile([128, 2, 65], F32R, tag="ctxT")
                    for mi in range(2):
                        pt = ps_o.tile([128, 65], F32, tag="po")
                        nc.tensor.transpose(pt[:], ctxs[:, 128 * mi:128 * (mi + 1)],
                                            ident_sb[0:65, 0:65])
                        nc.vector.tensor_copy(ctxT[:, mi, :], pt[:])

                    for j in range(J):
                        qpt = qpt_p.tile([128, 2, 512], F32R, tag="qpt")
                        pq = ps_q.tile([128, 2, 512], F32, tag="pq")
                        for mi in range(2):
                            nc.tensor.matmul(
                                pq[:, mi, :],
                                projT2_sb[DH:128, 128 * mi:128 * (mi + 1)],
                                qkT_sb[DH:128, 512 * j:512 * (j + 1)],
                                start=True, stop=True, skip_group_check=True)
                        nc.scalar.activation(qpt[:], pq[:], AF.Exp, scale=DS)
                        po = ps_o.tile([65, 512], F32, tag="po")
                        for mi in range(2):
                            nc.tensor.matmul(
                                po[:], ctxT[:, mi, :],
                                qpt[:, mi, :],
                                start=(mi == 0), stop=(mi == 1), skip_group_check=True)
                        stag = stag_p.tile([65, 512], F32, tag="stag")
                        nc.vector.tensor_copy(stag[:], po[:])
                        nc.sync.dma_start(out=h_in[b, j, DH * h:DH * (h + 1), :],
                                          in_=stag[0:DH, :])
                        nc.sync.dma_start(out=h_in[b, j, 128 + h:129 + h, :],
                                          in_=stag[DH:DH + 1, :])

                nc.gpsimd.collective_compute(
                    "AllToAll", ALU.bypass, replica_groups=groups,
                    ins=[h_in[b]], outs=[h_out[b]])

                DF = 2 * NCORES * NS // 128
                nc.sync.dma_start(
                    out=den_scr[b].rearrange("(s h n) -> s h n", s=NCORES, h=2),
                    in_=h_out[b, :, 128:130, :])
                den128 = small_p.tile([128, DF], F32, tag="den128")
                nc.sync.dma_start(
                    out=den128[:], in_=den_scr[b].rearrange("(p f) -> p f", f=DF))
                dinv128 = small_p.tile([128, DF], F32, tag="dinv128")
                nc.vector.reciprocal(dinv128[:], den128[:])
                nc.sync.dma_start(
                    out=dinv_scr[b].rearrange("(p f) -> p f", f=DF), in_=dinv128[:])

                hgn = lin_p.tile([128, NCORES, NS], F32R, tag="hgn")
                for cc in range(NCORES):
                    hraw = stag_p.tile([128, NS], F32, tag="hraw")
                    nc.sync.dma_start(out=hraw[:], in_=h_out[b, cc, 0:128, :])
                    dinvB = stag_p.tile([128, NS], F32, tag="dinvB")
                    nc.sync.dma_start(
                        out=dinvB[:],
                        in_=dinv_scr[b, cc * 2 * NS:(cc + 1) * 2 * NS]
                            .rearrange("(h n) -> h n", h=2)
                            .unsqueeze(1)
                            .broadcast_to([2, DH, NS]))
                    nc.gpsimd.tensor_tensor(out=hgn[:, cc, :], in0=hraw[:],
                                            in1=dinvB[:], op=ALU.mult)

                for nci in range(NS // 128):
                    for oh in range(2):
                        pl = ps_lin.tile([128, 512], F32, tag="pl")
                        for cc in range(NCORES):
                            nc.tensor.matmul(
                                pl[:],
                                hgn[:, cc, 128 * nci:128 * (nci + 1)],
                                WT_sb[:, cc, 512 * oh:512 * (oh + 1)],
                                start=(cc == 0), stop=(cc == NCORES - 1),
                                skip_group_check=True)
                        oc = outc_p.tile([128, 512], F32, tag="oc")
                        nc.scalar.activation(oc[:], pl[:], AF.Copy)
                        nc.sync.dma_start(
                            out=out_ext[b, 128 * nci:128 * (nci + 1),
                                        512 * oh:512 * (oh + 1)],
                            in_=oc[:])
    nc.compile()
    return nc


def _get_nc():
    if "nc" not in _CACHE:
        _CACHE["nc"] = _build()
    return _CACHE["nc"]


def _host_prep(q, k, v, W, proj):
    projT = np.ascontiguousarray(proj.T)
    projT2 = np.concatenate([projT, projT], axis=0)
    WTfull = np.ascontiguousarray(W.T).astype(np.float32)
    identity = np.eye(128, dtype=np.float32)
    in_maps = []
    for c in range(NCORES):
        lo = c * 128
        qc = q[:, :, lo:lo + 128]
        kc = k[:, :, lo:lo + 128]
        vc = v[:, :, lo:lo + 128]
        in_maps.append({
            "qT": np.ascontiguousarray(qc.reshape(B, N, 2, DH).transpose(0, 2, 3, 1)),
            "kT": np.ascontiguousarray(kc.reshape(B, N, 2, DH).transpose(0, 2, 3, 1)),
            "kn": np.ascontiguousarray(kc.reshape(B, T, 128, 128).transpose(0, 2, 1, 3)),
            "vn": np.ascontiguousarray(vc.reshape(B, T, 128, 128).transpose(0, 2, 1, 3)),
            "projT2": projT2,
            "WT": WTfull,
            "ident": identity,
        })
    return in_maps


def kernel(q, k, v, W, b, proj, _profile=False):
    q = np.asarray(q, np.float32)
    k = np.asarray(k, np.float32)
    v = np.asarray(v, np.float32)
    W = np.asarray(W, np.float32)
    b = np.asarray(b, np.float32)
    proj = np.asarray(proj, np.float32)

    nc = _get_nc()
    in_maps = _host_prep(q, k, v, W, proj)
    res = run_bass_kernel_spmd(nc, in_maps, list(range(NCORES)), trace=_profile)
    out = np.empty((B, N, D), dtype=np.float32)
    for c in range(NCORES):
        out[:, c * NS:(c + 1) * NS, :] = res.results[c]["out"]
    out += b
    if _profile:
        _CACHE["last_exec_time_ns"] = res.exec_time_ns
        _CACHE["last_profile_json"] = res.profile_json
    return out



# revision 9
# speedup vs baseline: 1.7741x; 1.7741x over previous
"""Performer (FAVOR+) multi-head fast-attention TRN2 kernel — self-contained.

Problem: B=4, N=4096, D=1024, H=16, M=256, DH=64.
Sharding: 2 heads per core (head-parallel attention) on 8 NeuronCores;
on-device AllToAll re-shards to sequence-parallel for the output Linear
(row-parallel, no partial sums); host stitches the 8 n-shards.

All matmul traffic is bf16 (PE runs 1 col/cycle vs 4 for fp32-HIGH);
accumulation stays fp32 in PSUM.  Stabilizers that cancel in the
num/den ratio are dropped; the k-side row max and ||k||^2 factors are
folded into v, matching the reference up to float rounding.

Software pipeline: per (b,h) AllToAll overlaps the next head's
attention; the output Linear of batch b-1 is interleaved into the
attention matmul stream of batch b so the PE never drains.
"""
import contextlib
import sys

sys.path.insert(0, "/opt/trn_rl_repo")

import numpy as np
import ml_dtypes

import concourse.bacc as bacc
import concourse.mybir as mybir
from concourse.tile import TileContext
from concourse.bass_utils import run_bass_kernel_spmd

F32 = mybir.dt.float32
BF16 = mybir.dt.bfloat16
AF = mybir.ActivationFunctionType
ALU = mybir.AluOpType
NPBF16 = ml_dtypes.bfloat16

NCORES = 8
B, N, D = 4, 4096, 1024
H, M, DH = 16, 256, 64
T = N // 128          # 32 token tiles of 128
J = N // 512          # 8 query blocks of 512
NS = N // NCORES      # 512 tokens per core after resharding
DS = float(DH) ** -0.25

_CACHE = {}


def _build():
    nc = bacc.Bacc(num_devices=NCORES)
    groups = [list(range(NCORES))]

    qkT = nc.declare_dram_parameter("qkT", [B, 2, 128, N], BF16, isOutput=False)
    knvn = nc.declare_dram_parameter("knvn", [B, 128, T, 256], BF16, isOutput=False)
    projT2 = nc.declare_dram_parameter("projT2", [128, M], BF16, isOutput=False)
    WT = nc.declare_dram_parameter("WT", [128, NCORES, D], BF16, isOutput=False)
    ident = nc.declare_dram_parameter("ident", [65, 65], F32, isOutput=False)
    out_ext = nc.declare_dram_parameter("out", [B, NS, D], F32, isOutput=True)

    h_in = nc.dram_tensor("h_in", [B, 2, NCORES, 65, NS], BF16)
    h_out = nc.dram_tensor("h_out", [B, 2, NCORES, 65, NS], BF16)
    dinv_scr = nc.dram_tensor("dinv_scr", [B, 16, NS], F32)
    warm_in = nc.dram_tensor("warm_in", [NCORES, 64], BF16)
    warm_out = nc.dram_tensor("warm_out", [NCORES, 64], BF16)

    with TileContext(nc) as tc:
        with contextlib.ExitStack() as stk:
            const_p = stk.enter_context(tc.tile_pool(name="const", bufs=1))
            qkT_p = stk.enter_context(tc.tile_pool(name="qkT", bufs=2))
            knvn_p = stk.enter_context(tc.tile_pool(name="knvn", bufs=2))
            ek_p = stk.enter_context(tc.tile_pool(name="ek", bufs=2))
            small_p = stk.enter_context(tc.tile_pool(name="small", bufs=3))
            vaug_p = stk.enter_context(tc.tile_pool(name="vaug", bufs=2))
            qpt_p = stk.enter_context(tc.tile_pool(name="qpt", bufs=3))
            stg_p = stk.enter_context(tc.tile_pool(name="stg", bufs=2))
            hx_p = stk.enter_context(tc.tile_pool(name="hx", bufs=1))
            oc_p = stk.enter_context(tc.tile_pool(name="oc", bufs=2))
            ps_feat = stk.enter_context(tc.tile_pool(name="psfeat", bufs=2, space="PSUM"))
            ps_cp = stk.enter_context(tc.tile_pool(name="pscp", bufs=1, space="PSUM"))
            ps_po = stk.enter_context(tc.tile_pool(name="pspo", bufs=2, space="PSUM"))
            ps_pl = stk.enter_context(tc.tile_pool(name="pspl", bufs=1, space="PSUM"))

            # -- warmup collective: primes the CC rings / absorbs core skew
            warm_sb = const_p.tile([NCORES, 64], BF16, tag="warm")
            nc.gpsimd.memset(warm_sb[:], 0.0)
            nc.sync.dma_start(out=warm_in[:], in_=warm_sb[:])
            nc.gpsimd.collective_compute(
                "AllToAll", ALU.bypass, replica_groups=groups,
                ins=[warm_in[:]], outs=[warm_out[:]])

            # -- constants
            projT2_sb = const_p.tile([128, M], BF16, tag="projT2")
            nc.sync.dma_start(out=projT2_sb[:], in_=projT2[:])
            ident_sb = const_p.tile([65, 65], F32, tag="ident")
            nc.sync.dma_start(out=ident_sb[:], in_=ident[:])
            WT_sb = const_p.tile([128, NCORES, D], BF16, tag="WT")
            nc.sync.dma_start(out=WT_sb[:], in_=WT[:])

            # ---- deferred emitters for the software pipeline ----
            state = {}

            def emit_knvn_load(b):
                knvn_sb = knvn_p.tile([128, T, 256], BF16, tag="knvn")
                nc.sync.dma_start(out=knvn_sb[:], in_=knvn[b])
                state[("knvn", b)] = knvn_sb

            def emit_ksq_dn(b):
                knvn_sb = state[("knvn", b)]
                ksl = knvn_sb[:, :, 0:128]
                nc.gpsimd.tensor_tensor(out=ksl, in0=ksl, in1=ksl, op=ALU.mult)
                dn_raw = small_p.tile([128, T, 2], F32, tag="dn")
                nc.vector.tensor_reduce(
                    out=dn_raw[:],
                    in_=knvn_sb[:, :, 0:128].rearrange("p t (h d) -> p t h d", h=2),
                    axis=mybir.AxisListType.X, op=ALU.add)
                state[("dn", b)] = dn_raw

            def emit_qkT_load(b, h):
                qkT_sb = qkT_p.tile([128, N], BF16, tag="qkT")
                nc.sync.dma_start(out=qkT_sb[:], in_=qkT[b, h])
                state[("qkT", b, h)] = qkT_sb

            def emit_post_dma(b):
                # After both AllToAlls of batch b: fetch numerators + dens,
                # build 1/den broadcast, scale -> hgn ready for the Linear.
                hraw = hx_p.tile([128, NCORES, NS], BF16, tag="hraw")
                for hh in range(2):
                    nc.sync.dma_start(
                        out=hraw[DH * hh:DH * (hh + 1), :, :],
                        in_=h_out[b, hh, :, 0:DH, :].rearrange("c d n -> d c n"))
                den16 = small_p.tile([16, NS], BF16, tag="den16")
                nc.sync.dma_start(
                    out=den16[:],
                    in_=h_out[b, :, :, DH, :].rearrange("h c n -> (h c) n"))
                dinv16 = small_p.tile([16, NS], F32, tag="dinv16")
                nc.vector.reciprocal(dinv16[:], den16[:])
                nc.sync.dma_start(out=dinv_scr[b], in_=dinv16[:])
                dinvB = hx_p.tile([128, NCORES, NS], F32, tag="dinvB")
                for hh in range(2):
                    nc.sync.dma_start(
                        out=dinvB[DH * hh:DH * (hh + 1), :, :],
                        in_=dinv_scr[b, 8 * hh:8 * (hh + 1), :]
                            .unsqueeze(0).broadcast_to([DH, NCORES, NS]))
                hgn = hx_p.tile([128, NCORES, NS], BF16, tag="hgn")
                nc.vector.tensor_tensor(out=hgn[:], in0=hraw[:], in1=dinvB[:],
                                        op=ALU.mult)
                state[("hgn", b)] = hgn

            def emit_lin_group(b, g):
                # one PSUM accumulation group of the output Linear of batch b
                hgn = state[("hgn", b)]
                nci, oh = g // 2, g % 2
                if oh == 0:
                    oc_new = oc_p.tile([128, 2, 512], F32, tag="oc", name="oc")
                    state[("oc", b, nci)] = oc_new
                oc = state[("oc", b, nci)]
                pl = ps_pl.tile([128, 512], F32, tag="pl")
                for cc in range(NCORES):
                    nc.tensor.matmul(
                        pl[:], hgn[:, cc, 128 * nci:128 * (nci + 1)],
                        WT_sb[:, cc, 512 * oh:512 * (oh + 1)],
                        start=(cc == 0), stop=(cc == NCORES - 1),
                        skip_group_check=True)
                nc.vector.tensor_copy(oc[:, oh, :], pl[:])
                if oh == 1:
                    nc.sync.dma_start(
                        out=out_ext[b, 128 * nci:128 * (nci + 1), :],
                        in_=oc[:].rearrange("p a f -> p (a f)"))

            def emit_head(b, h):
                # hooks used to interleave prior-batch post/linear work
                qkT_sb = state[("qkT", b, h)]
                knvn_sb = state[("knvn", b)]
                dn_raw = state[("dn", b)]

                # prefetch next head's qk tile
                if h == 0:
                    emit_qkT_load(b, 1)
                elif b + 1 < B:
                    emit_qkT_load(b + 1, 0)

                eg = small_p.tile([128, T], F32, tag="eg")
                nc.scalar.activation(eg[:], dn_raw[:, :, h], AF.Exp,
                                     scale=-0.5 * DS * DS)

                ek = ek_p.tile([128, T, M], BF16, tag="ek")
                me = small_p.tile([128, T], F32, tag="me")
                for tb in range(T // 4):
                    pf = ps_feat.tile([128, 2, 512], F32, tag="feat")
                    pf4 = pf[:].rearrange("p a (c f) -> p (a c) f", c=2)
                    for qq in range(4):
                        t = 4 * tb + qq
                        nc.tensor.matmul(
                            pf4[:, qq, :], qkT_sb[0:DH, 128 * t:128 * (t + 1)],
                            projT2_sb[0:DH, :],
                            start=True, stop=True, skip_group_check=True)
                    nc.scalar.activation(ek[:, 4 * tb:4 * (tb + 1), :], pf4[:],
                                         AF.Exp, scale=DS)
                    nc.vector.tensor_reduce(
                        out=me[:, 4 * tb:4 * (tb + 1)],
                        in_=ek[:, 4 * tb:4 * (tb + 1), :],
                        axis=mybir.AxisListType.X, op=ALU.max)
                    if h == 1 and tb == 0 and b > 0:
                        emit_post_dma(b - 1)

                rme = small_p.tile([128, T], F32, tag="rme")
                nc.vector.reciprocal(rme[:], me[:])
                g_t = small_p.tile([128, T], BF16, tag="g")
                nc.vector.tensor_tensor(out=g_t[:], in0=eg[:], in1=rme[:],
                                        op=ALU.mult)

                vaug = vaug_p.tile([128, T, 65], BF16, tag="vaug")
                nc.gpsimd.tensor_tensor(
                    out=vaug[:, :, 0:DH],
                    in0=knvn_sb[:, :, 128 + DH * h:128 + DH * (h + 1)],
                    in1=g_t[:].rearrange("p (t one) -> p t one", one=1)
                        .broadcast_to([128, T, DH]),
                    op=ALU.mult)
                nc.gpsimd.tensor_copy(vaug[:, :, DH], g_t[:])

                # A2A trigger for the PREVIOUS head slot goes on the gpsimd
                # queue here, after this head's vaug (so its sem wait can't
                # starve the vaug consumer).
                if h == 1:
                    nc.gpsimd.collective_compute(
                        "AllToAll", ALU.bypass, replica_groups=groups,
                        ins=[h_in[b, 0]], outs=[h_out[b, 0]])
                elif b > 0:
                    nc.gpsimd.collective_compute(
                        "AllToAll", ALU.bypass, replica_groups=groups,
                        ins=[h_in[b - 1, 1]], outs=[h_out[b - 1, 1]])

                cp = ps_cp.tile([128, 386], F32, tag="cp")
                pctx = cp[0:65, 0:256]
                for t in range(T):
                    nc.tensor.matmul(
                        pctx, vaug[:, t, :], ek[:, t, :],
                        start=(t == 0), stop=(t == T - 1), skip_group_check=True)
                ctxs = small_p.tile([65, 256], F32, tag="ctxs")
                nc.vector.tensor_copy(ctxs[:], pctx)

                ctxT = small_p.tile([128, 2, 65], BF16, tag="ctxT")
                for mi in range(2):
                    ptv = cp[:, 256 + 65 * mi:256 + 65 * (mi + 1)]
                    nc.tensor.transpose(ptv, ctxs[:, 128 * mi:128 * (mi + 1)],
                                        ident_sb[:])
                    nc.vector.tensor_copy(ctxT[:, mi, :], ptv)

                stg = stg_p.tile([65, J, 512], BF16, tag="stg")
                for j in range(J):
                    pf = ps_feat.tile([128, 2, 512], F32, tag="feat")
                    for mi in range(2):
                        nc.tensor.matmul(
                            pf[:, mi, :],
                            projT2_sb[DH:128, 128 * mi:128 * (mi + 1)],
                            qkT_sb[DH:128, 512 * j:512 * (j + 1)],
                            start=True, stop=True, skip_group_check=True)
                    qpt = qpt_p.tile([128, 2, 512], BF16, tag="qpt")
                    nc.scalar.activation(qpt[:], pf[:], AF.Exp, scale=DS)
                    po = ps_po.tile([65, 512], F32, tag="po")
                    for mi in range(2):
                        nc.tensor.matmul(
                            po[:], ctxT[:, mi, :], qpt[:, mi, :],
                            start=(mi == 0), stop=(mi == 1), skip_group_check=True)
                    nc.vector.tensor_copy(stg[:, j, :], po[:])
                    if h == 1 and b > 0 and j >= 3:
                        emit_lin_group(b - 1, j - 3)

                nc.sync.dma_start(
                    out=h_in[b, h].rearrange("c p n -> p c n"), in_=stg[:])
                if h == 1 and b > 0:
                    for g in range(5, 8):
                        emit_lin_group(b - 1, g)

            # ---- main pipeline ----
            emit_knvn_load(0)
            emit_qkT_load(0, 0)
            for b in range(B):
                emit_ksq_dn(b)
                if b + 1 < B:
                    emit_knvn_load(b + 1)
                for h in range(2):
                    emit_head(b, h)

            # tail: last AllToAll, last post + Linear
            nc.gpsimd.collective_compute(
                "AllToAll", ALU.bypass, replica_groups=groups,
                ins=[h_in[B - 1, 1]], outs=[h_out[B - 1, 1]])
            emit_post_dma(B - 1)
            for g in range(8):
                emit_lin_group(B - 1, g)

    nc.compile()
    return nc


def _get_nc():
    if "nc" not in _CACHE:
        _CACHE["nc"] = _build()
    return _CACHE["nc"]


def _host_prep(q, k, v, W):
    qb = q.astype(NPBF16)
    kb = k.astype(NPBF16)
    vb = v.astype(NPBF16)
    # W.T rearranged: WT[p, cc, o] = W[o, cc*128 + p]
    WTh = np.ascontiguousarray(
        W.T.astype(NPBF16).reshape(NCORES, 128, D).transpose(1, 0, 2))
    identity = np.eye(65, dtype=np.float32)
    in_maps = []
    for c in range(NCORES):
        lo = c * 128
        qc = qb[:, :, lo:lo + 128]   # [B, N, 128]
        kc = kb[:, :, lo:lo + 128]
        vc = vb[:, :, lo:lo + 128]
        # [B, 2, 64, N] transposed per head-pair
        kT = kc.reshape(B, N, 2, DH).transpose(0, 2, 3, 1)
        qT = qc.reshape(B, N, 2, DH).transpose(0, 2, 3, 1)
        qkTh = np.ascontiguousarray(
            np.concatenate([kT, qT], axis=2))   # [B, 2, 128, N]
        kn = kc.reshape(B, T, 128, 128).transpose(0, 2, 1, 3)
        vn = vc.reshape(B, T, 128, 128).transpose(0, 2, 1, 3)
        knvnh = np.ascontiguousarray(
            np.concatenate([kn, vn], axis=3))   # [B, 128, T, 256]
        in_maps.append({
            "qkT": qkTh,
            "knvn": knvnh,
            "projT2": None,   # filled below (shared)
            "WT": WTh,
            "ident": identity,
        })
    return in_maps


def kernel(q, k, v, W, b, proj, _profile=False):
    q = np.asarray(q, np.float32)
    k = np.asarray(k, np.float32)
    v = np.asarray(v, np.float32)
    W = np.asarray(W, np.float32)
    b = np.asarray(b, np.float32)
    proj = np.asarray(proj, np.float32)

    nc = _get_nc()
    in_maps = _host_prep(q, k, v, W)
    projT = np.ascontiguousarray(proj.T.astype(NPBF16))      # [64, M]
    projT2 = np.concatenate([projT, projT], axis=0)          # [128, M]
    for m in in_maps:
        m["projT2"] = projT2
    res = run_bass_kernel_spmd(nc, in_maps, list(range(NCORES)), trace=_profile)
    out = np.empty((B, N, D), dtype=np.float32)
    for c in range(NCORES):
        out[:, c * NS:(c + 1) * NS, :] = res.results[c]["out"]
    out += b
    if _profile:
        _CACHE["last_exec_time_ns"] = res.exec_time_ns
        _CACHE["last_profile_json"] = res.profile_json
    return out


# revision 28
# speedup vs baseline: 1.9488x; 1.0985x over previous
"""Performer (FAVOR+) multi-head fast-attention TRN2 kernel — self-contained.

Problem: B=4, N=4096, D=1024, H=16, M=256, DH=64.
Sharding: 2 heads per core (head-parallel attention) on 8 NeuronCores;
on-device AllToAll re-shards to sequence-parallel for the output Linear
(row-parallel, no partial sums); host stitches the 8 n-shards.

All matmul traffic is bf16 (PE runs 1 col/cycle vs 4 for fp32-HIGH);
accumulation stays fp32 in PSUM.  Stabilizers that cancel in the
num/den ratio are dropped; the k-side row max and ||k||^2 factors are
folded into v, matching the reference up to float rounding.

Pipeline: the kernel runs as 8 "slots" (one per batch x head-pair).
The scalar engine (exp) is the binding resource, so each slot emits
its own k-feature exps first and the PREVIOUS slot's q-feature exps
second, keeping scalar 100% busy.  ctx/out matmuls of the previous
slot fill the PE pacing gaps; the per-slot AllToAll and the output
Linear of earlier batches are interleaved behind.
"""
import contextlib
import sys

sys.path.insert(0, "/opt/trn_rl_repo")

import numpy as np
import ml_dtypes

import concourse.bacc as bacc
import concourse.mybir as mybir
from concourse.tile import TileContext
from concourse.bass_utils import run_bass_kernel_spmd

F32 = mybir.dt.float32
BF16 = mybir.dt.bfloat16
AF = mybir.ActivationFunctionType
ALU = mybir.AluOpType
NPBF16 = ml_dtypes.bfloat16

NCORES = 8
B, N, D = 4, 4096, 1024
H, M, DH = 16, 256, 64
T = N // 128          # 32 token tiles of 128
J = N // 512          # 8 query blocks of 512
NS = N // NCORES      # 512 tokens per core after resharding
DS = float(DH) ** -0.25

_CACHE = {}


def _build():
    nc = bacc.Bacc(num_devices=NCORES)
    groups = [list(range(NCORES))]

    qkT = nc.declare_dram_parameter("qkT", [B, 2, 128, N], BF16, isOutput=False)
    knvn = nc.declare_dram_parameter("knvn", [B, 128, T, 256], BF16, isOutput=False)
    projT2 = nc.declare_dram_parameter("projT2", [128, M], BF16, isOutput=False)
    WT = nc.declare_dram_parameter("WT", [128, NCORES, D], BF16, isOutput=False)
    ident = nc.declare_dram_parameter("ident", [65, 65], F32, isOutput=False)
    out_ext = nc.declare_dram_parameter("out", [B, NS, D], F32, isOutput=True)

    h_in = nc.dram_tensor("h_in", [B, 2, NCORES, 65, NS], BF16)
    h_out = nc.dram_tensor("h_out", [B, 2, NCORES, 65, NS], BF16)
    dinv_scr = nc.dram_tensor("dinv_scr", [B, 16, NS], F32)
    warm_in = nc.dram_tensor("warm_in", [NCORES, 64], BF16)
    warm_out = nc.dram_tensor("warm_out", [NCORES, 64], BF16)

    with TileContext(nc) as tc:
        with contextlib.ExitStack() as stk:
            const_p = stk.enter_context(tc.tile_pool(name="const", bufs=1))
            qkT_p = stk.enter_context(tc.tile_pool(name="qkT", bufs=3))
            knvn_p = stk.enter_context(tc.tile_pool(name="knvn", bufs=2))
            ek_p = stk.enter_context(tc.tile_pool(name="ek", bufs=2))
            small_p = stk.enter_context(tc.tile_pool(name="small", bufs=3))
            vaug_p = stk.enter_context(tc.tile_pool(name="vaug", bufs=2))
            qpt_p = stk.enter_context(tc.tile_pool(name="qpt", bufs=4))
            stg_p = stk.enter_context(tc.tile_pool(name="stg", bufs=2))
            hx_p = stk.enter_context(tc.tile_pool(name="hx", bufs=1))
            oc_p = stk.enter_context(tc.tile_pool(name="oc", bufs=2))
            # PSUM budget (8 banks): feat 2x2 + pq 2 + cpo 1 + pl 1.
            # cpo holds the ctx accumulator early-slot and the out-MM
            # bank late-slot (sequential uses); transposes borrow feat
            # tiles.
            ps_feat = stk.enter_context(tc.tile_pool(name="psfeat", bufs=2, space="PSUM"))
            ps_pq = stk.enter_context(tc.tile_pool(name="pspq", bufs=1, space="PSUM"))
            ps_pl = stk.enter_context(tc.tile_pool(name="pspl", bufs=1, space="PSUM"))
            ps_cpo = stk.enter_context(tc.tile_pool(name="pscpo", bufs=1, space="PSUM"))

            # -- warmup collective: primes the CC rings / absorbs core skew
            warm_sb = const_p.tile([NCORES, 64], BF16, tag="warm")
            nc.gpsimd.memset(warm_sb[:], 0.0)
            nc.sync.dma_start(out=warm_in[:], in_=warm_sb[:])
            nc.gpsimd.collective_compute(
                "AllToAll", ALU.bypass, replica_groups=groups,
                ins=[warm_in[:]], outs=[warm_out[:]])

            # -- constants
            projT2_sb = const_p.tile([128, M], BF16, tag="projT2")
            nc.sync.dma_start(out=projT2_sb[:], in_=projT2[:])
            ident_sb = const_p.tile([65, 65], F32, tag="ident")
            nc.sync.dma_start(out=ident_sb[:], in_=ident[:])
            WT_sb = const_p.tile([128, NCORES, D], BF16, tag="WT")
            nc.sync.dma_start(out=WT_sb[:], in_=WT[:])

            pq = ps_pq.tile([128, 2, 512], F32, tag="pq")
            cpo = ps_cpo.tile([65, 512], F32, tag="cpo")

            state = {}

            def emit_knvn_load(b):
                knvn_sb = knvn_p.tile([128, T, 256], BF16, tag="knvn")
                nc.sync.dma_start(out=knvn_sb[:], in_=knvn[b])
                state[("knvn", b)] = knvn_sb

            def emit_ksq_dn(b):
                knvn_sb = state[("knvn", b)]
                ksl = knvn_sb[:, :, 0:128]
                nc.gpsimd.tensor_tensor(out=ksl, in0=ksl, in1=ksl, op=ALU.mult)
                dn_raw = small_p.tile([128, T, 2], F32, tag="dn")
                nc.vector.tensor_reduce(
                    out=dn_raw[:],
                    in_=knvn_sb[:, :, 0:128].rearrange("p t (h d) -> p t h d", h=2),
                    axis=mybir.AxisListType.X, op=ALU.add)
                state[("dn", b)] = dn_raw

            def emit_qkT_load(b, h):
                qkT_sb = qkT_p.tile([128, N], BF16, tag="qkT")
                nc.sync.dma_start(out=qkT_sb[:], in_=qkT[b, h])
                state[("qkT", b, h)] = qkT_sb

            def emit_post_dma(b):
                # After both AllToAlls of batch b: fetch numerators + dens,
                # build 1/den broadcast, scale -> hgn ready for the Linear.
                hraw = hx_p.tile([128, NCORES, NS], BF16, tag="hraw")
                for hh in range(2):
                    nc.sync.dma_start(
                        out=hraw[DH * hh:DH * (hh + 1), :, :],
                        in_=h_out[b, hh, :, 0:DH, :].rearrange("c d n -> d c n"))
                den16 = small_p.tile([16, NS], BF16, tag="den16")
                nc.sync.dma_start(
                    out=den16[:],
                    in_=h_out[b, :, :, DH, :].rearrange("h c n -> (h c) n"))
                dinv16 = small_p.tile([16, NS], F32, tag="dinv16")
                nc.vector.reciprocal(dinv16[:], den16[:])
                nc.sync.dma_start(out=dinv_scr[b], in_=dinv16[:])
                dinvB = hx_p.tile([128, NCORES, NS], F32, tag="dinvB")
                for hh in range(2):
                    nc.sync.dma_start(
                        out=dinvB[DH * hh:DH * (hh + 1), :, :],
                        in_=dinv_scr[b, 8 * hh:8 * (hh + 1), :]
                            .unsqueeze(0).broadcast_to([DH, NCORES, NS]))
                hgn = hx_p.tile([128, NCORES, NS], BF16, tag="hgn")
                nc.gpsimd.tensor_tensor(out=hgn[:], in0=hraw[:], in1=dinvB[:],
                                        op=ALU.mult)
                state[("hgn", b)] = hgn

            def emit_lin_group(b, g):
                # one PSUM accumulation group of the output Linear of batch b
                hgn = state[("hgn", b)]
                nci, oh = g // 2, g % 2
                if oh == 0:
                    oc_new = oc_p.tile([128, 2, 512], F32, tag="oc", name="oc")
                    state[("oc", b, nci)] = oc_new
                oc = state[("oc", b, nci)]
                pl = ps_pl.tile([128, 512], F32, tag="pl")
                for cc in range(NCORES):
                    nc.tensor.matmul(
                        pl[:], hgn[:, cc, 128 * nci:128 * (nci + 1)],
                        WT_sb[:, cc, 512 * oh:512 * (oh + 1)],
                        start=(cc == 0), stop=(cc == NCORES - 1),
                        skip_group_check=True)
                nc.vector.tensor_copy(oc[:, oh, :], pl[:])
                if oh == 1:
                    nc.sync.dma_start(
                        out=out_ext[b, 128 * nci:128 * (nci + 1), :],
                        in_=oc[:].rearrange("p a f -> p (a f)"))

            # ---- per-slot pieces -------------------------------------
            def emit_kf_step(s, tb):
                # 4 k-feature MMs into a feat tile + exp + me chain
                b, h = s
                qkT_sb = state[("qkT", b, h)]
                ek = state[("ek", s)]
                me = state[("me", s)]
                pf = ps_feat.tile([128, 2, 512], F32, tag="feat", name="pf")
                pf4 = pf[:].rearrange("p a (c f) -> p (a c) f", c=2)
                for qq in range(4):
                    t = 4 * tb + qq
                    nc.tensor.matmul(
                        pf4[:, qq, :], qkT_sb[0:DH, 128 * t:128 * (t + 1)],
                        projT2_sb[0:DH, :],
                        start=True, stop=True, skip_group_check=True)
                nc.scalar.activation(
                    ek[:, 4 * tb:4 * (tb + 1), :], pf4[:], AF.Exp, scale=DS)
                nc.vector.tensor_reduce(
                    out=me[:, 4 * tb:4 * (tb + 1)],
                    in_=ek[:, 4 * tb:4 * (tb + 1), :],
                    axis=mybir.AxisListType.X, op=ALU.max)
                if tb == 7:
                    emit_gq(s)

            def emit_gq(s):
                # g = exp(-dn') / me, then vaug = [v * g | g]
                b, h = s
                knvn_sb = state[("knvn", b)]
                me = state[("me", s)]
                eg = state[("eg", s)]
                rme = state[("rme", s)]
                g_t = state[("g", s)]
                vaug = state[("vaug", s)]
                nc.vector.reciprocal(rme[:], me[:])
                nc.vector.tensor_tensor(
                    out=g_t[:], in0=eg[:], in1=rme[:], op=ALU.mult)
                nc.gpsimd.tensor_tensor(
                    out=vaug[:, :, 0:DH],
                    in0=knvn_sb[:, :, 128 + DH * h:128 + DH * (h + 1)],
                    in1=g_t[:].rearrange("p (t one) -> p t one", one=1)
                        .broadcast_to([128, T, DH]),
                    op=ALU.mult)
                nc.gpsimd.tensor_copy(vaug[:, :, DH], g_t[:])

            def emit_ctx_group(s, gi):
                # 4 ctx accumulation MMs (tiles 4*gi..4*gi+3) of slot s
                ek = state[("ek", s)]
                vaug = state[("vaug", s)]
                for t in range(4 * gi, 4 * gi + 4):
                    nc.tensor.matmul(
                        cpo[:, 0:256], vaug[:, t, :], ek[:, t, :],
                        start=(t == 0), stop=(t == T - 1), skip_group_check=True)

            def emit_trans(s):
                ctxs = small_p.tile([65, 256], F32, tag="ctxs")
                nc.vector.tensor_copy(ctxs[:], cpo[:, 0:256])
                ctxT = small_p.tile([128, 2, 65], BF16, tag="ctxT")
                pf_t = ps_feat.tile([128, 2, 512], F32, tag="feat", name="pf_t")
                for mi in range(2):
                    ptv = pf_t[:, mi, 0:65]
                    nc.tensor.transpose(ptv, ctxs[:, 128 * mi:128 * (mi + 1)],
                                        ident_sb[:])
                    nc.vector.tensor_copy(ctxT[:, mi, :], ptv)
                state[("ctxT", s)] = ctxT

            def emit_qf(s, j):
                # q-feature MMs + exp for block j of slot s
                b, h = s
                qkT_sb = state[("qkT", b, h)]
                for mi in range(2):
                    nc.tensor.matmul(
                        pq[:, mi, :],
                        projT2_sb[DH:128, 128 * mi:128 * (mi + 1)],
                        qkT_sb[DH:128, 512 * j:512 * (j + 1)],
                        start=True, stop=True, skip_group_check=True)
                qpt = qpt_p.tile([128, 2, 512], BF16, tag="qpt")
                nc.scalar.activation(qpt[:], pq[:], AF.Exp, scale=DS)
                state[("qpt", s, j)] = qpt

            def emit_out(s, j):
                b, h = s
                ctxT = state[("ctxT", s)]
                qpt = state.pop(("qpt", s, j))
                stg = state[("stg", s)]
                for mi in range(2):
                    nc.tensor.matmul(
                        cpo[:], ctxT[:, mi, :], qpt[:, mi, :],
                        start=(mi == 0), stop=(mi == 1), skip_group_check=True)
                nc.vector.tensor_copy(stg[:, j, :], cpo[:])

            def emit_slot_open(s):
                b, h = s
                ek = ek_p.tile([128, T, M], BF16, tag="ek", name="ek")
                state[("ek", s)] = ek
                state[("me", s)] = small_p.tile([128, T], BF16, tag="me", name="me")
                state[("rme", s)] = small_p.tile([128, T], F32, tag="rme", name="rme")
                g_new = small_p.tile([128, T], BF16, tag="g", name="g")
                state[("g", s)] = g_new
                vaug = vaug_p.tile([128, T, 65], BF16, tag="vaug", name="vaug")
                state[("vaug", s)] = vaug
                eg = small_p.tile([128, T], F32, tag="eg", name="eg")
                dn_raw = state[("dn", b)]
                nc.scalar.activation(eg[:], dn_raw[:, :, h], AF.Exp,
                                     scale=-0.5 * DS * DS)
                state[("eg", s)] = eg

            def emit_slot_close_prev(prev):
                # stg of prev is complete: ship it + trigger its AllToAll
                b, h = prev
                stg = state[("stg", prev)]
                nc.sync.dma_start(
                    out=h_in[b, h].rearrange("c p n -> p c n"), in_=stg[:])
                nc.gpsimd.collective_compute(
                    "AllToAll", ALU.bypass, replica_groups=groups,
                    ins=[h_in[b, h]], outs=[h_out[b, h]])

            # ---- slot schedule ---------------------------------------
            # slot sigma = 2b + h.  In slot sigma we emit:
            #   kF(sigma) [scalar: ek exps], ctx(sigma-1), trans(sigma-1),
            #   qF(sigma-1) [scalar: qpt exps], out(sigma-1), lin hooks.
            slots = [(b, h) for b in range(B) for h in range(2)]

            def lin_hooks_for(sigma):
                # linear groups of batch bb become available at slot 2bb+4
                for bb in range(B):
                    if sigma == 2 * bb + 4:
                        return [(bb, g) for g in range(8)]
                return []

            emit_knvn_load(0)
            emit_qkT_load(0, 0)

            for sigma, s in enumerate(slots):
                b, h = s
                prev = slots[sigma - 1] if sigma > 0 else None
                if h == 0:
                    emit_ksq_dn(b)
                    if b + 1 < B:
                        emit_knvn_load(b + 1)
                emit_slot_open(s)
                # prefetch next slot's qk tile
                if sigma + 1 < len(slots):
                    emit_qkT_load(*slots[sigma + 1])
                if prev is not None:
                    state[("stg", prev)] = stg_p.tile(
                        [65, J, 512], BF16, tag="stg", name="stg")
                lins = lin_hooks_for(sigma)

                # interleave: kF steps + prev ctx groups + first prev qF
                for tb in range(8):
                    emit_kf_step(s, tb)
                    if prev is not None:
                        if tb >= 2:
                            emit_ctx_group(prev, tb - 2)
                        if tb == 5:
                            emit_qf(prev, 0)
                        if tb == 6:
                            emit_qf(prev, 1)
                    if tb == 7 and b > 0 and h == 1:
                        # A2A(b-1, 1) done by now -> build hgn(b-1)
                        emit_post_dma(b - 1)
                if prev is not None:
                    emit_ctx_group(prev, 6)
                    emit_ctx_group(prev, 7)
                    emit_trans(prev)
                    for j in range(J):
                        if j >= 2:
                            emit_qf(prev, j)
                        emit_out(prev, j)
                        if lins and j < len(lins):
                            emit_lin_group(*lins[j])
                    emit_slot_close_prev(prev)

            # ---- tail: last slot's ctx/q/out + final linear ----------
            last = slots[-1]
            state[("stg", last)] = stg_p.tile([65, J, 512], BF16, tag="stg",
                                              name="stg")
            for gi in range(8):
                emit_ctx_group(last, gi)
            emit_trans(last)
            lins_t = [(2, g) for g in range(8)]
            for j in range(J):
                emit_qf(last, j)
                emit_out(last, j)
                emit_lin_group(*lins_t[j])
            emit_slot_close_prev(last)
            emit_post_dma(B - 1)
            for g in range(8):
                emit_lin_group(B - 1, g)

    nc.compile()
    return nc


def _get_nc():
    if "nc" not in _CACHE:
        _CACHE["nc"] = _build()
    return _CACHE["nc"]


def _host_prep(q, k, v, W):
    qb = q.astype(NPBF16)
    kb = k.astype(NPBF16)
    vb = v.astype(NPBF16)
    # W.T rearranged: WT[p, cc, o] = W[o, cc*128 + p]
    WTh = np.ascontiguousarray(
        W.T.astype(NPBF16).reshape(NCORES, 128, D).transpose(1, 0, 2))
    identity = np.eye(65, dtype=np.float32)
    in_maps = []
    for c in range(NCORES):
        lo = c * 128
        qc = qb[:, :, lo:lo + 128]   # [B, N, 128]
        kc = kb[:, :, lo:lo + 128]
        vc = vb[:, :, lo:lo + 128]
        # [B, 2, 64, N] transposed per head-pair
        kT = kc.reshape(B, N, 2, DH).transpose(0, 2, 3, 1)
        qT = qc.reshape(B, N, 2, DH).transpose(0, 2, 3, 1)
        qkTh = np.ascontiguousarray(
            np.concatenate([kT, qT], axis=2))   # [B, 2, 128, N]
        kn = kc.reshape(B, T, 128, 128).transpose(0, 2, 1, 3)
        vn = vc.reshape(B, T, 128, 128).transpose(0, 2, 1, 3)
        knvnh = np.ascontiguousarray(
            np.concatenate([kn, vn], axis=3))   # [B, 128, T, 256]
        in_maps.append({
            "qkT": qkTh,
            "knvn": knvnh,
            "projT2": None,   # filled below (shared)
            "WT": WTh,
            "ident": identity,
        })
    return in_maps


def kernel(q, k, v, W, b, proj, _profile=False):
    q = np.asarray(q, np.float32)
    k = np.asarray(k, np.float32)
    v = np.asarray(v, np.float32)
    W = np.asarray(W, np.float32)
    b = np.asarray(b, np.float32)
    proj = np.asarray(proj, np.float32)

    nc = _get_nc()
    in_maps = _host_prep(q, k, v, W)
    projT = np.ascontiguousarray(proj.T.astype(NPBF16))      # [64, M]
    projT2 = np.concatenate([projT, projT], axis=0)          # [128, M]
    for m in in_maps:
        m["projT2"] = projT2
    res = run_bass_kernel_spmd(nc, in_maps, list(range(NCORES)), trace=_profile)
    out = np.empty((B, N, D), dtype=np.float32)
    for c in range(NCORES):
        out[:, c * NS:(c + 1) * NS, :] = res.results[c]["out"]
    out += b
    if _profile:
        _CACHE["last_exec_time_ns"] = res.exec_time_ns
        _CACHE["last_profile_json"] = res.profile_json
    return out
